# revision 28
# baseline (speedup 1.0000x reference)
"""HCNN (known-U) recurrence kernel for 8 Trainium2 NeuronCores.

Model (see reference): 80 sequential steps of
    state' = tanh(cat(post_state, u)) @ A            A: (2112, 2048) fp32
with teacher forcing post_state[:, :128] = y during the 64 past steps,
outputs = 64 past errors then 16 forecasts (first 128 state components).

Strategy
--------
Data-parallel over batch: 256 = 8 cores x 32. Each core runs the full
recurrence for its batch slice; no collectives.

Per-core per-step matmul x @ A with batch M=32 would waste 3/4 of the
128-wide PE array, so the A columns are split into 4 interleaved groups
and computed by 4 concurrent column-tiled matmuls (tile_position=(0,32j))
sharing the array. Data is fp16 (single pass): the teacher-forced
recurrence is strongly contractive, measured end-to-end output error
~1.5e-4 relative vs the fp32 reference.

Column interleave: state column s lives in col-group j=(s//32)%4 at free
offset 32*(s//128) + s%32. With that mapping the psum holding state'
(batch on partitions within each 32-group) turns into the next step's
stationary operand layout via a DVE 32x32 block-transpose: block (j, m')
is internally transposed in place, landing the operand exactly where
matmul k-tile m' reads it.

Step pipelining (the key perf structure): each step accumulates into TWO
half-width psum tiles in separate banks -- ps_lo (free cols 0:256, state
cols 0:1024) over 17 rounds of N=256, then ps_hi (cols 256:512). ps_lo
finalizes mid-step, so its tanh (ACT) + block-transpose (DVE) run
concurrently with the hi-half matmuls; ps_hi's tanh/transpose are split
in two 128-col chunks whose first chunk is ready before the next step's
k=8 round needs it. Net: the inter-step tanh+transpose chain is mostly
off the PE critical path, in the past AND forecast regimes (forecast
k=0 reads the lo transpose, which is ready before the step ends).
y/u/init contributions are pre-tanh'ed and pre-transposed on the host;
per-step round order is [y, u, state tiles]. The last step only computes
the 128 output columns (N=32). Output slices DMA out in 16-step chunks.

Two measured-on-HW fixes worth keeping: (1) per-matmul completion sem
increments serialize at ~34 ns on the EVT path and throttle the whole
stream at 136 MMs/step -- _thin_pe_incs keeps 1 inc per 4-MM round and
rescales every wait threshold to ceil(v/4); (2) ~4 us of dummy matmuls
gated on the first A tile flip the HAM clock gate to 8/8 mid-load so
post-load steps run at 2.4 GHz. Keeping each half's 17 rounds contiguous
in the PE stream matters: interleaving lo/hi phases (tried twice) adds
scattered no-wait PE stalls costing more than the latency margin gained.
"""

import sys

for _p in ("/opt/trn_rl_repo", "/root/.axon_site/_ro/trn_rl_repo"):
    if _p not in sys.path:
        sys.path.insert(0, _p)

import numpy as np

N_STATE = 2048
N_U = 64
N_Y = 128
PAST = 64
FORE = 16
BATCH = 256
T = PAST + FORE          # 80 total steps; only 79 matmul steps needed
NSTEP = T - 1            # step t computes state_{t+1}; state_80 is unused
NK = 17                  # contraction tiles: 16 x 128 state + 1 x (64 u + 64 pad)
KDIM = NK * 128          # 2176 padded contraction size
N_CORES = 8
B = BATCH // N_CORES     # 32 per core
OUT_CHUNK = 16           # steps per output DMA chunk


def _build_program():
    import concourse.bass as bass
    import concourse.tile as tile
    from concourse import mybir

    F32 = mybir.dt.float32
    F16 = mybir.dt.float16

    nc = bass.Bass("TRN2", target_bir_lowering=False, debug=False,
                   num_devices=N_CORES)

    A_ext = nc.declare_dram_parameter("A_re", [KDIM, 4, 512], F16, isOutput=False)
    ytanhT_ext = nc.declare_dram_parameter("ytanhT", [128, PAST * B], F16, isOutput=False)
    utanhT_ext = nc.declare_dram_parameter("utanhT", [128, NSTEP * B], F16, isOutput=False)
    ywrap_ext = nc.declare_dram_parameter("ywrap", [128, (PAST - 1) * B], F16, isOutput=False)
    initxT_ext = nc.declare_dram_parameter("initxT", [128, 512], F16, isOutput=False)
    out_ext = nc.declare_dram_parameter("outbuf", [128, NSTEP * B], F16, isOutput=True)

    korder = [0, 16] + list(range(1, 16))
    # hi-half tanh/transpose chunk widths (cols): chunk 0 is transposed in
    # time for the next step's k=8 round (~1.1 us after hi stop). Keeping
    # each half's 17 rounds contiguous matters: interleaving lo/hi phases
    # (tried twice) adds scattered no-wait PE stalls that cost more than
    # the dependency margin it buys.
    HI_CHUNKS = [(0, 128), (128, 128)]

    with tile.TileContext(nc) as tc:
        with tc.tile_pool(name="const", bufs=1) as cpool, \
             tc.tile_pool(name="xbuf", bufs=2) as xpool, \
             tc.tile_pool(name="th", bufs=2) as thpool, \
             tc.tile_pool(name="psum", bufs=2, space="PSUM") as pspool:

            # Small inputs first: step rounds k=0 (y) / k=16 (u) and the
            # init state must not wait behind the 8.9 MB A stream.
            ytanhT = cpool.tile([128, PAST * B], F16, tag="yt")
            nc.sync.dma_start(out=ytanhT[:], in_=ytanhT_ext[:])
            utanhT = cpool.tile([128, NSTEP * B], F16, tag="ut")
            nc.sync.dma_start(out=utanhT[:], in_=utanhT_ext[:])

            xlo = xpool.tile([128, 256], F16, tag="xlo")
            xhi = xpool.tile([128, 256], F16, tag="xhi")
            nc.sync.dma_start(out=xlo[:], in_=initxT_ext[:, 0:256])
            nc.sync.dma_start(out=xhi[:], in_=initxT_ext[:, 256:512])

            ywrap = cpool.tile([128, (PAST - 1) * B], F16, tag="yw")
            nc.sync.dma_start(out=ywrap[:], in_=ywrap_ext[:])

            A_sb = cpool.tile([128, NK * 2048], F16, tag="A")
            for k in korder:
                nc.sync.dma_start(out=A_sb[:, 2048 * k:2048 * (k + 1)],
                                  in_=A_ext[128 * k:128 * (k + 1), :, :])

            outbuf = cpool.tile([128, NSTEP * B], F16, tag="ob")

            def lhs_for(t, k, lo, hi):
                if k == 0:
                    if t < PAST:
                        return ytanhT[:, B * t:B * (t + 1)]
                    return lo[:, 0:32]
                if k == 16:
                    return utanhT[:, B * t:B * (t + 1)]
                if k < 8:
                    return lo[:, 32 * k:32 * (k + 1)]
                return hi[:, 32 * (k - 8):32 * (k - 7)]

            # Dummy rounds gated on the first A tile. All 17 A-tile DMAs run
            # concurrently and complete near the END of the ~25 us load, so
            # without filler the PE idles >3.4 us mid-load and the HAM clock
            # gate re-throttles to 4/8 (measured: warm 21->26 us, cold to
            # 45 us). ~104 rounds = ~20 us of PE activity spans the load:
            # warm from ~18 us through the first real steps at 2.4 GHz.
            ps_w = pspool.tile([128, 512], F32, tag="warm")
            for w in range(104):
                nc.tensor.matmul(ps_w[0:32, :], xlo[:, 0:32],
                                 A_sb[:, 512 * (w % 4):512 * (w % 4 + 1)],
                                 start=True, stop=True,
                                 tile_position=(0, 0))

            def mm_rounds(t, ps, ks, off, width, start, stop):
                for idx, k in enumerate(ks):
                    lhsT = lhs_for(t, k, xlo, xhi)
                    for j in range(4):
                        base = 2048 * k + 512 * j + off
                        nc.tensor.matmul(
                            ps[32 * j:32 * (j + 1), :],
                            lhsT, A_sb[:, base:base + width],
                            start=start and idx == 0,
                            stop=stop and idx == len(ks) - 1,
                            tile_position=(0, 32 * j),
                        )

            for t in range(NSTEP):
                last = t == NSTEP - 1
                # Last step: only the 128 output cols (psum free cols 0:32)
                # are ever read, and there is no next state to build.
                NL = 32 if last else 256
                ps_lo = pspool.tile([128, NL], F32, tag="pslo")
                mm_rounds(t, ps_lo, korder, 0, NL, True, True)

                if not last:
                    # Runs on ACT/DVE while the hi-half matmuls stream.
                    th_lo = thpool.tile([128, 256], F16, tag="thlo")
                    nc.scalar.activation(th_lo[:], ps_lo[:],
                                         mybir.ActivationFunctionType.Tanh)
                    nlo = xpool.tile([128, 256], F16, tag="xlo")
                    nc.vector.transpose(nlo[:], th_lo[:])

                # Output slot t: expectation = state' cols 0:128 = ps_lo[:, 0:32].
                # Emitted after the tanh so the same-bank psum access order
                # keeps tanh first; both run off the critical path.
                if t + 1 < PAST:
                    nc.vector.tensor_sub(outbuf[:, B * t:B * (t + 1)],
                                         ps_lo[:, 0:32],
                                         ywrap[:, B * t:B * (t + 1)])
                else:
                    nc.vector.tensor_copy(outbuf[:, B * t:B * (t + 1)],
                                          ps_lo[:, 0:32])

                if not last:
                    ps_hi = pspool.tile([128, 256], F32, tag="pshi")
                    mm_rounds(t, ps_hi, korder, 256, 256, True, True)
                    # Chunked so the first transposed piece lands before the
                    # next step's k=8 round needs it (~1.1 us after hi stop).
                    th_hi = thpool.tile([128, 256], F16, tag="thhi")
                    nhi = xpool.tile([128, 256], F16, tag="xhi")
                    for c0, w in HI_CHUNKS:
                        sl = slice(c0, c0 + w)
                        nc.scalar.activation(th_hi[:, sl], ps_hi[:, sl],
                                             mybir.ActivationFunctionType.Tanh)
                        nc.vector.transpose(nhi[:, sl], th_hi[:, sl])
                    xlo, xhi = nlo, nhi

                if (t + 1) % OUT_CHUNK == 0 or last:
                    c0 = (t // OUT_CHUNK) * OUT_CHUNK
                    nc.sync.dma_start(out=out_ext[:, B * c0:B * (t + 1)],
                                      in_=outbuf[:, B * c0:B * (t + 1)])

    _thin_pe_incs(nc)
    _split_multi_waits(nc)
    return nc


def _thin_pe_incs(nc, group=4):
    """Per-MM completion sem increments serialize at ~34 ns on the EVT
    write path; at 136 matmuls/step they throttle the whole PE stream
    (measured: the sem counter ticks at exactly 1/34 ns regardless of
    matmul timing). Walrus requires UpdateValue == 1, so: keep only every
    `group`-th matmul's increment (each round is 4 concurrent col-tiled
    MMs completing in pc order) and rescale every wait threshold v on
    those sems to ceil(v/group). A rescaled wait unblocks at the first
    kept increment at old-count >= v, i.e. never earlier than before."""

    def _walk():
        for f in nc.m.functions:
            for b in f.blocks:
                for ins in b.instructions:
                    yield ins

    # Sems incremented by matmuls, and by nothing else.
    mm_sems = set()
    for ins in _walk():
        si = ins.sync_info
        if si is None:
            continue
        for u in si.on_update:
            if u.sync_type == 'semaphore' and u.update_mode == 'sem-inc':
                if type(ins).__name__ == 'InstMatmult':
                    mm_sems.add(u.id)
    for ins in _walk():
        si = ins.sync_info
        if si is None or type(ins).__name__ == 'InstMatmult':
            continue
        for u in si.on_update:
            if u.sync_type == 'semaphore' and u.id in mm_sems:
                mm_sems.discard(u.id)   # someone else touches it: leave alone
    if not mm_sems:
        return

    # Sanity: only plain immediate-ge waits reference these sems.
    for ins in _walk():
        si = ins.sync_info
        if si is None:
            continue
        for w in si.on_wait:
            if getattr(w, 'id', None) in mm_sems:
                assert w.wait_mode == 'sem-ge-imm' and w.wait_reg is None, \
                    f"unexpected wait on thinned sem: {w}"

    # Drop increments: keep every group-th (and the final one if ragged).
    nseen = dict.fromkeys(mm_sems, 0)
    last_inc_ins = dict.fromkeys(mm_sems)
    for ins in _walk():
        if type(ins).__name__ != 'InstMatmult':
            continue
        si = ins.sync_info
        if si is None or not si.on_update:
            continue
        keep = []
        changed = False
        for u in si.on_update:
            if (u.sync_type == 'semaphore' and u.id in mm_sems
                    and u.update_mode == 'sem-inc'):
                assert u.update_value == 1
                nseen[u.id] += 1
                if nseen[u.id] % group == 0:
                    keep.append(u)
                    last_inc_ins[u.id] = None
                else:
                    changed = True
                    last_inc_ins[u.id] = (ins, u)
            else:
                keep.append(u)
        if changed:
            import concourse.mybir as mybir
            ins.sync_info = mybir.SyncInfo(
                on_wait=list(si.on_wait), on_update=keep)
    import concourse.mybir as mybir
    for sid, tail in last_inc_ins.items():
        if tail is not None:          # ragged tail: restore final increment
            ins, u = tail
            si = ins.sync_info
            ins.sync_info = mybir.SyncInfo(
                on_wait=list(si.on_wait), on_update=list(si.on_update) + [u])
            nseen[sid] = nseen[sid] // group + 1   # kept count bookkeeping

    # Rescale all wait thresholds on the thinned sems.
    for ins in _walk():
        si = ins.sync_info
        if si is None:
            continue
        if not any(getattr(w, 'id', None) in mm_sems for w in si.on_wait):
            continue
        new_waits = []
        for w in si.on_wait:
            if getattr(w, 'id', None) in mm_sems:
                new_waits.append(mybir.SyncWait(
                    sync_type=w.sync_type, id=w.id, ant_name=w.ant_name,
                    wait_mode=w.wait_mode,
                    wait_value=-(-w.wait_value // group),
                    wait_reg=None))
            else:
                new_waits.append(w)
        ins.sync_info = mybir.SyncInfo(
            on_wait=new_waits, on_update=list(si.on_update))


def _split_multi_waits(nc):
    """This walrus build accepts at most one sem wait per instruction; Tile
    sometimes emits more. Hoist extras onto nops inserted just before the
    instruction in the same engine stream."""
    from concourse import mybir

    n = 0
    for f in nc.m.functions:
        for b in f.blocks:
            insts = b.instructions
            out = []
            changed = False
            for ins in insts:
                si = ins.sync_info
                if si is not None and len(si.on_wait) > 1:
                    waits = list(si.on_wait)
                    for w in waits[:-1]:
                        n += 1
                        out.append(mybir.InstNoOp(
                            name=f"I-waitsplit-{n}",
                            engine=ins.engine,
                            ins=[], outs=[],
                            bass_nofuse=True,
                            sync_info=mybir.SyncInfo(on_wait=[w], on_update=[]),
                        ))
                    ins.sync_info = mybir.SyncInfo(
                        on_wait=[waits[-1]], on_update=list(si.on_update))
                    changed = True
                out.append(ins)
            if changed:
                b.instructions = out


def _host_inputs(U, Y, A, init_state):
    """Build the per-core input maps (all pre-tanh / pre-transpose work)."""
    A = np.asarray(A, np.float32)
    U = np.asarray(U, np.float32)
    Y = np.asarray(Y, np.float32)
    init_state = np.asarray(init_state, np.float32)

    A_pad = np.zeros((KDIM, N_STATE), np.float16)
    A_pad[:N_STATE + N_U] = A.astype(np.float16)
    # column interleave: col s -> (j=(s//32)%4, free 32*(s//128)+s%32)
    A_re = np.ascontiguousarray(
        A_pad.reshape(KDIM, 16, 4, 32).transpose(0, 2, 1, 3).reshape(KDIM, 4, 512))

    init_tanh = np.tanh(init_state[0]).astype(np.float16)          # (2048,)
    initxT = np.ascontiguousarray(
        np.broadcast_to(init_tanh.reshape(16, 128).T[:, None, :].transpose(0, 2, 1),
                        (128, 16, 32)).reshape(128, 512))

    ytanh = np.tanh(Y).astype(np.float16)                          # (64, 256, 128)
    utanh = np.tanh(U[:NSTEP]).astype(np.float16)                  # (79, 256, 64)

    in_maps = []
    for c in range(N_CORES):
        b0 = c * B
        yt = np.ascontiguousarray(
            ytanh[:, b0:b0 + B, :].transpose(0, 2, 1)              # (64, 128, 32)
            .transpose(1, 0, 2).reshape(128, PAST * B))
        ut = np.zeros((128, NSTEP * B), np.float16)
        ut[:N_U] = (utanh[:, b0:b0 + B, :].transpose(0, 2, 1)      # (79, 64, 32)
                    .transpose(1, 0, 2).reshape(N_U, NSTEP * B))
        # ywrap slot s (=1..63) at cols 32*(s-1): rows 32j+b = Y[s, b0+b, 32j+cc]
        yw = (Y[1:PAST, b0:b0 + B, :].reshape(PAST - 1, B, 4, 32)
              .transpose(0, 2, 1, 3)                               # (63, 4, 32b, 32cc)
              .reshape(PAST - 1, 128, 32)
              .transpose(1, 0, 2).reshape(128, (PAST - 1) * B))
        in_maps.append({
            "A_re": A_re,
            "ytanhT": yt,
            "utanhT": np.ascontiguousarray(ut),
            "ywrap": np.ascontiguousarray(yw.astype(np.float16)),
            "initxT": initxT,
        })
    return in_maps


def kernel(U, Y, A, init_state):
    from concourse.bass_utils import run_bass_kernel_spmd

    nc = _build_program()
    in_maps = _host_inputs(U, Y, A, init_state)
    res = run_bass_kernel_spmd(nc, in_maps, list(range(N_CORES)))

    out = np.empty((T, BATCH, N_Y), np.float32)
    # slot 0: err for t=0 is pure host math (state_0 = broadcast init_state)
    out[0] = np.asarray(init_state, np.float32)[0, :N_Y][None, :] - np.asarray(Y, np.float32)[0]
    for c in range(N_CORES):
        b0 = c * B
        ob = np.asarray(res.results[c]["outbuf"], np.float32)      # (128, 79*32)
        # [32j+b, 32t+cc] = out[t+1, b0+b, 32j+cc]
        ob4 = ob.reshape(4, 32, NSTEP, 32)                         # (j, b, t, cc)
        out[1:, b0:b0 + B, :] = ob4.transpose(2, 1, 0, 3).reshape(NSTEP, B, N_Y)
    return out


if __name__ == "__main__":
    rng = np.random.default_rng(0)
    U = rng.standard_normal((T, BATCH, N_U)).astype(np.float32)
    Y = rng.standard_normal((PAST, BATCH, N_Y)).astype(np.float32)
    A = (rng.standard_normal((N_STATE + N_U, N_STATE)) * 0.02).astype(np.float32)
    init = rng.standard_normal((1, N_STATE)).astype(np.float32)
    o = kernel(U=U, Y=Y, A=A, init_state=init)
    print("kernel out:", o.shape, o.dtype)
